# revision 6
# baseline (speedup 1.0000x reference)
"""AdderNet CNN forward on 8 TRN2 NeuronCores — pure data parallel over batch.

Reference computation per layer l (8 layers):
  y[b,o,h,w] = -sum_{c,kh,kw} |x[b,c,h+kh-1,w+kw-1] - w[o,c,kh,kw]|   (zero pad)
  x' = relu(s[o]*y + bias[o])
maxpool 2x2 after layers 2, 4, 8; then flatten -> Linear(2048, 10).

Strategy per core (16 images):
  - activations live in SBUF as [channel_partition, (b, Hpad, Wpad)] bf16 with
    zeroed 1-px borders, so conv taps are free-dim offsets
  - im2col: SBUF->SBUF DMA builds compact patch tiles [128 taps, pix]
  - per (o, patch tile): |x-w| = relu(x-w) - min(x-w, 0):
      DVE path: two 4x-mode tensor_scalar ops (add/max, add/min vs -w[o, taps])
      ACT path: one 1x activation(Abs, bias=-w)  (routes a fraction of o's)
  - TensorE reduces over taps: matmul with a +/-1 basis-column lhsT so the
    result lands in psum row o%32 (col group o//32); accumulate over tiles
  - epilogue: one ACT Relu(-s*psum + bias) -> next layer (or pool tmp)
  - FC: 16 accumulated matmuls [128c,10] x [128c,16b] -> psum[10,16] + bias
"""
import numpy as np

B_TOTAL = 128
N_CORES = 8
BC = B_TOTAL // N_CORES  # 16 images per core

# (O, C, Hin, pool_after)
LAYERS = [
    (32, 3, 32, False),
    (32, 32, 32, True),
    (64, 32, 16, False),
    (64, 64, 16, True),
    (128, 64, 8, False),
    (128, 128, 8, False),
    (128, 128, 8, False),
    (128, 128, 8, True),
]

_CACHE = {}


def _build(cfg=None):
    from contextlib import ExitStack
    import concourse.bacc as bacc
    import concourse.bass as bass
    import concourse.mybir as mybir
    import concourse.tile as tile

    cfg = dict(cfg or {})
    act_mod = cfg.get("act_mod", 8)       # o%act_mod < act_cnt -> ACT path
    act_cnt = cfg.get("act_cnt", 3)
    loop_k = cfg.get("loop_k", 0)         # >0: wrap whole net in For_i (timing)

    f32, bf16 = mybir.dt.float32, mybir.dt.bfloat16
    A = mybir.AluOpType
    AF = mybir.ActivationFunctionType

    nc = bacc.Bacc("TRN2", target_bir_lowering=False, debug=False)

    x_d = nc.dram_tensor("x", [BC, 3, 32, 32], f32, kind="ExternalInput")
    w_d, s_d, b_d = {}, {}, {}
    for i, (O, C, H, _) in enumerate(LAYERS):
        w_d[i] = nc.dram_tensor(f"w{i+1}", [O, C, 3, 3], f32, kind="ExternalInput")
        s_d[i] = nc.dram_tensor(f"s{i+1}", [O], f32, kind="ExternalInput")
        b_d[i] = nc.dram_tensor(f"b{i+1}", [O], f32, kind="ExternalInput")
    fcw_d = nc.dram_tensor("fc_w", [10, 2048], f32, kind="ExternalInput")
    fcb_d = nc.dram_tensor("fc_b", [10], f32, kind="ExternalInput")
    out_d = nc.dram_tensor("out", [BC, 10], f32, kind="ExternalOutput")

    with tile.TileContext(nc) as tc, ExitStack() as ctx:
        persist = ctx.enter_context(tc.tile_pool(name="persist", bufs=1))
        wpool = ctx.enter_context(tc.tile_pool(name="wpool", bufs=1))
        dpool = ctx.enter_context(tc.tile_pool(name="dpool", bufs=6))
        pspool = ctx.enter_context(tc.tile_pool(name="pspool", bufs=2, space="PSUM"))

        # padded activation tensors, channel-partition, (b, H+2, W+2) free
        Ap = []  # entry i: input to layer i
        shapes = []
        for i, (O, C, H, _) in enumerate(LAYERS):
            shapes.append((C, H))
        for i, (C, H) in enumerate(shapes):
            t = persist.tile([C, BC * (H + 2) * (H + 2)], bf16, name=f"Ap{i}",
                             tag=f"Ap{i}")
            nc.vector.memset(t, 0.0)
            Ap.append(t)
        A8 = persist.tile([128, BC * 16], bf16, name="A8", tag="A8")  # FC input

        # basis tensors: Tpos/Tneg [128, 64], column 32 = +/-1
        Tpos = persist.tile([128, 64], bf16, name="Tpos", tag="Tpos")
        Tneg = persist.tile([128, 64], bf16, name="Tneg", tag="Tneg")
        nc.vector.memset(Tpos, 0.0)
        nc.vector.memset(Tneg, 0.0)
        nc.vector.memset(Tpos[:, 32:33], 1.0)
        nc.vector.memset(Tneg[:, 32:33], -1.0)

        # load input x -> Ap[0] interior (f32 -> bf16), in 4-image chunks
        a0v = Ap[0].rearrange("c (b h w) -> c b h w", b=BC, h=34)
        with tc.tile_pool(name="xload", bufs=2) as xpool:
            for g in range(4):
                xs = xpool.tile([3, 4 * 1024], f32, name=f"xs{g}", tag="xs")
                nc.sync.dma_start(out=xs, in_=bass.AP(
                    tensor=x_d, offset=g * 4 * 3 * 1024,
                    ap=[[1024, 3], [3 * 1024, 4], [1, 1024]]))
                nc.vector.tensor_copy(
                    a0v[:, g * 4:(g + 1) * 4, 1:33, 1:33],
                    xs.rearrange("c (b h w) -> c b h w", b=4, h=32))

        # per-layer weights: wneg[t] [rows, O] f32 = -w[o, f], f = blk*C + c
        wneg_all, negs_all, bb_all = [], [], []
        for i, (O, C, H, _) in enumerate(LAYERS):
            CKK = C * 9
            T = (CKK + 127) // 128
            wneg_l = []
            for t in range(T):
                rows = min(128, CKK - t * 128)
                wtmp = wpool.tile([rows, O], f32, name=f"wtmp{i}_{t}", tag="wtmp",
                                  bufs=2)
                # fill by (dh,dw) block
                blk0 = t * 128 // C
                nblk = rows // C
                for bi in range(nblk):
                    blk = blk0 + bi
                    nc.sync.dma_start(
                        out=wtmp[bi * C:(bi + 1) * C, :],
                        in_=bass.AP(tensor=w_d[i], offset=blk,
                                    ap=[[9, C], [C * 9, O]]))
                wn = wpool.tile([rows, O], f32, name=f"wneg{i}_{t}",
                                tag=f"wneg{i}_{t}")
                nc.vector.tensor_scalar_mul(wn, wtmp, -1.0)
                wneg_l.append(wn)
            wneg_all.append(wneg_l)

            st = wpool.tile([O, 1], f32, name=f"st{i}", tag="st_tmp", bufs=2)
            nc.sync.dma_start(out=st, in_=bass.AP(tensor=s_d[i], offset=0,
                                                  ap=[[1, O], [1, 1]]))
            ns = wpool.tile([O, 1], f32, name=f"negs{i}", tag=f"negs{i}")
            nc.vector.tensor_scalar_mul(ns, st, -1.0)
            negs_all.append(ns)
            bb = wpool.tile([O, 1], f32, name=f"bb{i}", tag=f"bb{i}")
            nc.sync.dma_start(out=bb, in_=bass.AP(tensor=b_d[i], offset=0,
                                                  ap=[[1, O], [1, 1]]))
            bb_all.append(bb)

        # FC weights [c, (hw, cls)] bf16 and bias [10, 1] f32
        fcs = persist.tile([128, 160], f32, name="fcs", tag="fcs")
        nc.sync.dma_start(out=fcs, in_=bass.AP(
            tensor=fcw_d, offset=0, ap=[[16, 128], [1, 16], [2048, 10]]))
        fcw = persist.tile([128, 160], bf16, name="fcw", tag="fcw")
        nc.vector.tensor_copy(fcw, fcs)
        fcb = persist.tile([10, 1], f32, name="fcb", tag="fcb")
        nc.sync.dma_start(out=fcb, in_=bass.AP(tensor=fcb_d, offset=0,
                                               ap=[[1, 10], [1, 1]]))

        def net_body():
            for li, (O, C, H, pool_after) in enumerate(LAYERS):
                CKK = C * 9
                T = (CKK + 127) // 128
                Hp = H + 2
                W = H
                src = Ap[li]
                srcv = src.rearrange("c (b h w) -> c b h w", b=BC, h=Hp)
                nQ = max(1, O // 32)
                # batch chunking: big layers processed in halves
                n_bch = 4 if H == 32 else 1
                bcs = BC // n_bch
                npix_c = bcs * H * W
                # psum pix chunk (2 banks/tile, 2 bufs + FC tile <= 8 banks)
                pch = min(npix_c, 1024)

                with ExitStack() as lctx:
                    ppool = lctx.enter_context(
                        tc.tile_pool(name=f"patch{li}", bufs=1))
                    tpool = (lctx.enter_context(
                        tc.tile_pool(name=f"ptmp{li}", bufs=1))
                        if pool_after else None)

                    for bch in range(n_bch):
                        b0 = bch * bcs
                        # --- build patch tiles via SBUF->SBUF DMA ---
                        pt = []
                        for t in range(T):
                            rows = min(128, CKK - t * 128)
                            p = ppool.tile([rows, npix_c], bf16,
                                           name=f"p{li}_{bch}_{t}", tag=f"pt{t}")
                            pt.append(p)
                        for blk in range(9):
                            dh, dw = blk // 3, blk % 3
                            gr = blk * C
                            t, r0 = gr // 128, gr % 128
                            for bi in range(bcs):
                                nc.sync.dma_start(
                                    out=pt[t][r0:r0 + C,
                                              bi * H * W:(bi + 1) * H * W].rearrange(
                                        "c (h w) -> c h w", h=H),
                                    in_=srcv[0:C, b0 + bi, dh:dh + H, dw:dw + W])

                        if pool_after:
                            dest = tpool.tile([O, npix_c], bf16,
                                              name=f"tmp{li}_{bch}", tag="tmp")
                        # --- absdiff + PE reduce + epilogue, per psum chunk ---
                        for p0 in range(0, npix_c, pch):
                            ps = pspool.tile([max(32, O), pch], f32,
                                             name=f"ps{li}_{bch}_{p0}", tag="ps")
                            nsl = pch // 512
                            for j in range(32):
                                for q in range(nQ):
                                    o = q * 32 + j
                                    if o >= O:
                                        continue
                                    use_act = (o % act_mod) < act_cnt
                                    for t in range(T):
                                        rows = pt[t].shape[0]
                                        wcol = wneg_all[li][t][:, o:o + 1]
                                        first = (j == 0 and t == 0)
                                        last = (j == 31 or o == O - 1) and t == T - 1
                                        if use_act:
                                            d = dpool.tile([rows, pch], bf16,
                                                           name=f"d{li}", tag="d")
                                            nc.scalar.activation(
                                                d, pt[t][:, p0:p0 + pch], AF.Abs,
                                                bias=wcol, scale=1.0)
                                            for sl in range(nsl):
                                                nc.tensor.matmul(
                                                    ps[q * 32:q * 32 + 32,
                                                       sl * 512:(sl + 1) * 512],
                                                    Tpos[0:rows, 32 - j:64 - j],
                                                    d[:, sl * 512:(sl + 1) * 512],
                                                    start=first, stop=last,
                                                    tile_position=(0, 32 * q),
                                                    skip_group_check=True)
                                        else:
                                            r1 = dpool.tile([rows, pch], bf16,
                                                            name=f"r1{li}", tag="d")
                                            r2 = dpool.tile([rows, pch], bf16,
                                                            name=f"r2{li}", tag="d")
                                            nc.vector.tensor_scalar(
                                                r1, pt[t][:, p0:p0 + pch], wcol,
                                                0.0, A.add, A.max)
                                            nc.vector.tensor_scalar(
                                                r2, pt[t][:, p0:p0 + pch], wcol,
                                                0.0, A.add, A.min)
                                            for sl in range(nsl):
                                                nc.tensor.matmul(
                                                    ps[q * 32:q * 32 + 32,
                                                       sl * 512:(sl + 1) * 512],
                                                    Tpos[0:rows, 32 - j:64 - j],
                                                    r1[:, sl * 512:(sl + 1) * 512],
                                                    start=first, stop=False,
                                                    tile_position=(0, 32 * q),
                                                    skip_group_check=True)
                                                nc.tensor.matmul(
                                                    ps[q * 32:q * 32 + 32,
                                                       sl * 512:(sl + 1) * 512],
                                                    Tneg[0:rows, 32 - j:64 - j],
                                                    r2[:, sl * 512:(sl + 1) * 512],
                                                    start=False, stop=last,
                                                    tile_position=(0, 32 * q),
                                                    skip_group_check=True)
                            # epilogue: relu(-s * psum + b)
                            if pool_after:
                                nc.scalar.activation(
                                    dest[:, p0:p0 + pch], ps[0:O, :], AF.Relu,
                                    bias=bb_all[li][:, :], scale=negs_all[li][:, :])
                            else:
                                Hn = H  # same spatial size, next layer pad Hn+2
                                dv = Ap[li + 1].rearrange(
                                    "c (b h w) -> c b h w", b=BC, h=Hn + 2)
                                # pixel range [p0, p0+pch) within this bchunk:
                                # whole images per chunk (pch % (H*W) == 0)
                                i0 = b0 + p0 // (H * W)
                                ni = pch // (H * W)
                                nc.scalar.activation(
                                    dv[0:O, i0:i0 + ni, 1:H + 1, 1:W + 1],
                                    ps[0:O, :].rearrange(
                                        "c (b h w) -> c b h w", b=ni, h=H),
                                    AF.Relu,
                                    bias=bb_all[li][:, :], scale=negs_all[li][:, :])

                        # --- maxpool 2x2 -> next padded tensor (or A8) ---
                        if pool_after:
                            dv4 = dest.rearrange("c (b h w) -> c b h w", b=bcs, h=H)
                            m1 = tpool.tile([O, npix_c // 2], bf16,
                                            name=f"m1_{li}_{bch}", tag="m1")
                            m1v = m1.rearrange("c (b h w) -> c b h w", b=bcs, h=H)
                            nc.vector.tensor_tensor(
                                m1v, dv4[:, :, :, 0::2], dv4[:, :, :, 1::2], A.max)
                            if li == 7:
                                nxt = A8.rearrange("c (b h w) -> c b h w",
                                                   b=BC, h=4)[0:O, b0:b0 + bcs]
                            else:
                                Hn = H // 2
                                nxt = Ap[li + 1].rearrange(
                                    "c (b h w) -> c b h w", b=BC, h=Hn + 2)[
                                    0:O, b0:b0 + bcs, 1:Hn + 1, 1:Hn + 1]
                            nc.vector.tensor_tensor(
                                nxt, m1v[:, :, 0::2, :], m1v[:, :, 1::2, :], A.max)

            # --- FC: out[cls, b] = sum_c,hw A8[c, b*16+hw] * fcw[c, hw*10+cls] ---
            psf = pspool.tile([32, 512], f32, name="psf", tag="psf", bufs=1)
            for hw in range(16):
                nc.tensor.matmul(
                    psf[0:10, 0:BC],
                    fcw[:, hw * 10:(hw + 1) * 10],
                    bass.AP(tensor=A8.tensor, offset=A8.offset + hw,
                            ap=[list(A8.ap[0]), [16, BC]]),
                    start=(hw == 0), stop=(hw == 15), skip_group_check=True)
            osb = persist.tile([10, BC], f32, name="osb", tag="osb")
            nc.scalar.activation(osb, psf[0:10, 0:BC], AF.Identity,
                                 bias=fcb[:, :], scale=1.0)
            nc.sync.dma_start(
                out=bass.AP(tensor=out_d, offset=0, ap=[[1, 10], [10, BC]]),
                in_=osb)

        if loop_k > 1:
            with tc.For_i(0, loop_k, 1):
                net_body()
        else:
            net_body()

    nc.compile()
    return nc


def _get_nc(cfg=None):
    key = str(sorted((cfg or {}).items()))
    if key not in _CACHE:
        _CACHE[key] = _build(cfg)
    return _CACHE[key]


def make_in_maps(inputs):
    full = dict(inputs)
    x = np.ascontiguousarray(full["x"], dtype=np.float32)
    reps = {k: np.ascontiguousarray(v, np.float32) for k, v in full.items()
            if k != "x"}
    in_maps = []
    for c in range(N_CORES):
        m = {"x": x[c * BC:(c + 1) * BC]}
        m.update(reps)
        in_maps.append(m)
    return in_maps


def kernel(**inputs):
    from concourse import bass_utils
    nc = _get_nc()
    in_maps = make_in_maps(inputs)
    res = bass_utils.run_bass_kernel_spmd(nc, in_maps,
                                          core_ids=list(range(N_CORES)))
    return np.concatenate([r["out"] for r in res.results], axis=0)


# revision 9
# speedup vs baseline: 1.0666x; 1.0666x over previous
"""AdderNet CNN forward on 8 TRN2 NeuronCores — pure data parallel over batch.

Reference computation per layer l (8 layers):
  y[b,o,h,w] = -sum_{c,kh,kw} |x[b,c,h+kh-1,w+kw-1] - w[o,c,kh,kw]|   (zero pad)
  x' = relu(s[o]*y + bias[o])
maxpool 2x2 after layers 2, 4, 8; then flatten -> Linear(2048, 10).

Strategy per core (16 images):
  - activations live in SBUF as [channel_partition, (b, Hpad, Wpad)] bf16 with
    zeroed 1-px borders, so conv taps are free-dim offsets
  - im2col: SBUF->SBUF DMA builds compact patch tiles [128 taps, pix]
  - per (o, patch tile): |x-w| = relu(x-w) - min(x-w, 0):
      DVE path: two 4x-mode tensor_scalar ops (add/max, add/min vs -w[o, taps])
      ACT path: one 1x activation(Abs, bias=-w)  (routes a fraction of o's)
  - TensorE reduces over taps: matmul with a +/-1 basis-column lhsT so the
    result lands in psum row o%32 (col group o//32); accumulate over tiles
  - epilogue: one ACT Relu(-s*psum + bias) -> next layer (or pool tmp)
  - FC: 16 accumulated matmuls [128c,10] x [128c,16b] -> psum[10,16] + bias
"""
import numpy as np

B_TOTAL = 128
N_CORES = 8
BC = B_TOTAL // N_CORES  # 16 images per core

# (O, C, Hin, pool_after)
LAYERS = [
    (32, 3, 32, False),
    (32, 32, 32, True),
    (64, 32, 16, False),
    (64, 64, 16, True),
    (128, 64, 8, False),
    (128, 128, 8, False),
    (128, 128, 8, False),
    (128, 128, 8, True),
]

_CACHE = {}


def _build(cfg=None):
    from contextlib import ExitStack
    import concourse.bacc as bacc
    import concourse.bass as bass
    import concourse.mybir as mybir
    import concourse.tile as tile

    cfg = dict(cfg or {})
    loop_k = cfg.get("loop_k", 0)         # >0: wrap whole net in For_i (timing)

    # custom DVE ops: fused |x0-w0|+|x1-w1| (pair) and |x-w| (single)
    from concourse.dve_spec import Spec, Src0, Src1, C0, C1, maxx, lower, _has_src1
    from concourse.dve_uop import DveOpSpec
    from concourse import dve_ops

    def _register(name, spec):
        for o in dve_ops.OPS:
            if o.name == name:
                return o
        op = dve_ops.DveOp(name, spec, subdim=False, uops_sha={})
        dve_ops.OPS.append(op)
        dve_ops.CUSTOM_DVE_SPECS[name] = spec
        dve_ops._SUB_OPCODE_FOR_NAME[name] = (
            dve_ops._CUSTOM_DVE_ROW_BASE + len(dve_ops.OPS) - 1)
        for ver in ("v3", "v4"):
            s = DveOpSpec(name=name, opcode=dve_ops.get_dve_sub_opcode(name),
                          uops=lower(spec, ver=ver), rd1_en=_has_src1(spec))
            op.uops_sha[ver] = s.sha(ver)
        return op

    PAIRSAD = _register("PAIR_SAD_ANT", Spec(
        body=maxx(Src0 - C0, C0 - Src0) + maxx(Src1 - C1, C1 - Src1),
        reference=lambda in0, in1, s0, s1, imm2: (
            np.abs(in0.astype(np.float32) - np.asarray(s0, np.float32).reshape(-1, 1))
            + np.abs(in1.astype(np.float32) - np.asarray(s1, np.float32).reshape(-1, 1)))))
    ABSD = _register("ABS_DIFF_ANT", Spec(
        body=maxx(Src0 - C0, C0 - Src0),
        reference=lambda in0, in1, s0, s1, imm2: np.abs(
            in0.astype(np.float32) - np.asarray(s0, np.float32).reshape(-1, 1))))

    f32, bf16 = mybir.dt.float32, mybir.dt.bfloat16
    A = mybir.AluOpType
    AF = mybir.ActivationFunctionType

    nc = bacc.Bacc("TRN2", target_bir_lowering=False, debug=False)

    x_d = nc.dram_tensor("x", [BC, 3, 32, 32], f32, kind="ExternalInput")
    w_d, s_d, b_d = {}, {}, {}
    for i, (O, C, H, _) in enumerate(LAYERS):
        w_d[i] = nc.dram_tensor(f"w{i+1}", [O, C, 3, 3], f32, kind="ExternalInput")
        s_d[i] = nc.dram_tensor(f"s{i+1}", [O], f32, kind="ExternalInput")
        b_d[i] = nc.dram_tensor(f"b{i+1}", [O], f32, kind="ExternalInput")
    fcw_d = nc.dram_tensor("fc_w", [10, 2048], f32, kind="ExternalInput")
    fcb_d = nc.dram_tensor("fc_b", [10], f32, kind="ExternalInput")
    out_d = nc.dram_tensor("out", [BC, 10], f32, kind="ExternalOutput")

    with tile.TileContext(nc) as tc, ExitStack() as ctx:
        persist = ctx.enter_context(tc.tile_pool(name="persist", bufs=1))
        wpool = ctx.enter_context(tc.tile_pool(name="wpool", bufs=1))
        dpool = ctx.enter_context(tc.tile_pool(name="dpool", bufs=4))
        pspool = ctx.enter_context(tc.tile_pool(name="pspool", bufs=2, space="PSUM"))

        # padded activation tensors, channel-partition, (b, H+2, W+2) free
        Ap = []  # entry i: input to layer i
        shapes = []
        for i, (O, C, H, _) in enumerate(LAYERS):
            shapes.append((C, H))
        for i, (C, H) in enumerate(shapes):
            t = persist.tile([C, BC * (H + 2) * (H + 2)], bf16, name=f"Ap{i}",
                             tag=f"Ap{i}")
            nc.vector.memset(t, 0.0)
            Ap.append(t)
        A8 = persist.tile([128, BC * 16], bf16, name="A8", tag="A8")  # FC input

        # basis tensors: Tpos/Tneg [128, 64], column 32 = +/-1
        Tpos = persist.tile([128, 64], bf16, name="Tpos", tag="Tpos")
        Tneg = persist.tile([128, 64], bf16, name="Tneg", tag="Tneg")
        nc.vector.memset(Tpos, 0.0)
        nc.vector.memset(Tneg, 0.0)
        nc.vector.memset(Tpos[:, 32:33], 1.0)
        nc.vector.memset(Tneg[:, 32:33], -1.0)

        # load input x -> Ap[0] interior (f32 -> bf16), in 4-image chunks
        a0v = Ap[0].rearrange("c (b h w) -> c b h w", b=BC, h=34)
        with tc.tile_pool(name="xload", bufs=2) as xpool:
            for g in range(4):
                xs = xpool.tile([3, 4 * 1024], f32, name=f"xs{g}", tag="xs")
                nc.sync.dma_start(out=xs, in_=bass.AP(
                    tensor=x_d, offset=g * 4 * 3 * 1024,
                    ap=[[1024, 3], [3 * 1024, 4], [1, 1024]]))
                nc.vector.tensor_copy(
                    a0v[:, g * 4:(g + 1) * 4, 1:33, 1:33],
                    xs.rearrange("c (b h w) -> c b h w", b=4, h=32))

        # per-layer weights, f = blk*C + c:
        #   wpos[t] [rows, O] f32 = +w (custom-DVE ops), wneg = -w (ACT bias)
        wpos_all, wneg_all, negs_all, bb_all = [], [], [], []
        for i, (O, C, H, _) in enumerate(LAYERS):
            CKK = C * 9
            T = (CKK + 127) // 128
            wpos_l, wneg_l = [], []
            for t in range(T):
                rows = min(128, CKK - t * 128)
                wp = wpool.tile([rows, O], f32, name=f"wpos{i}_{t}",
                                tag=f"wpos{i}_{t}")
                blk0 = t * 128 // C
                nblk = rows // C
                for bi in range(nblk):
                    blk = blk0 + bi
                    nc.sync.dma_start(
                        out=wp[bi * C:(bi + 1) * C, :],
                        in_=bass.AP(tensor=w_d[i], offset=blk,
                                    ap=[[9, C], [C * 9, O]]))
                wn = wpool.tile([rows, O], f32, name=f"wneg{i}_{t}",
                                tag=f"wneg{i}_{t}")
                nc.vector.tensor_scalar_mul(wn, wp, -1.0)
                wpos_l.append(wp)
                wneg_l.append(wn)
            wpos_all.append(wpos_l)
            wneg_all.append(wneg_l)

            st = wpool.tile([O, 1], f32, name=f"st{i}", tag="st_tmp", bufs=2)
            nc.sync.dma_start(out=st, in_=bass.AP(tensor=s_d[i], offset=0,
                                                  ap=[[1, O], [1, 1]]))
            ns = wpool.tile([O, 1], f32, name=f"negs{i}", tag=f"negs{i}")
            nc.vector.tensor_scalar_mul(ns, st, -1.0)
            negs_all.append(ns)
            bb = wpool.tile([O, 1], f32, name=f"bb{i}", tag=f"bb{i}")
            nc.sync.dma_start(out=bb, in_=bass.AP(tensor=b_d[i], offset=0,
                                                  ap=[[1, O], [1, 1]]))
            bb_all.append(bb)

        # FC weights [c, (hw, cls)] bf16 and bias [10, 1] f32
        fcs = persist.tile([128, 160], f32, name="fcs", tag="fcs")
        nc.sync.dma_start(out=fcs, in_=bass.AP(
            tensor=fcw_d, offset=0, ap=[[16, 128], [1, 16], [2048, 10]]))
        fcw = persist.tile([128, 160], bf16, name="fcw", tag="fcw")
        nc.vector.tensor_copy(fcw, fcs)
        fcb = persist.tile([10, 1], f32, name="fcb", tag="fcb")
        nc.sync.dma_start(out=fcb, in_=bass.AP(tensor=fcb_d, offset=0,
                                               ap=[[1, 10], [1, 1]]))

        def net_body():
            for li, (O, C, H, pool_after) in enumerate(LAYERS):
                CKK = C * 9
                T = (CKK + 127) // 128
                Hp = H + 2
                W = H
                src = Ap[li]
                srcv = src.rearrange("c (b h w) -> c b h w", b=BC, h=Hp)
                nQ = max(1, O // 32)
                # batch chunking: big layers processed in halves
                n_bch = 4 if H == 32 else 1
                bcs = BC // n_bch
                npix_c = bcs * H * W
                # psum pix chunk (2 banks/tile, 2 bufs + FC tile <= 8 banks)
                pch = min(npix_c, 1024)

                with ExitStack() as lctx:
                    ppool = lctx.enter_context(
                        tc.tile_pool(name=f"patch{li}", bufs=1))
                    tpool = (lctx.enter_context(
                        tc.tile_pool(name=f"ptmp{li}", bufs=1))
                        if pool_after else None)

                    for bch in range(n_bch):
                        b0 = bch * bcs
                        # --- build patch tiles via SBUF->SBUF DMA ---
                        pt = []
                        for t in range(T):
                            rows = min(128, CKK - t * 128)
                            p = ppool.tile([rows, npix_c], bf16,
                                           name=f"p{li}_{bch}_{t}", tag=f"pt{t}")
                            pt.append(p)
                        for blk in range(9):
                            dh, dw = blk // 3, blk % 3
                            gr = blk * C
                            t, r0 = gr // 128, gr % 128
                            for bi in range(bcs):
                                nc.sync.dma_start(
                                    out=pt[t][r0:r0 + C,
                                              bi * H * W:(bi + 1) * H * W].rearrange(
                                        "c (h w) -> c h w", h=H),
                                    in_=srcv[0:C, b0 + bi, dh:dh + H, dw:dw + W])

                        if pool_after:
                            dest = tpool.tile([O, npix_c], bf16,
                                              name=f"tmp{li}_{bch}", tag="tmp")
                        # --- absdiff + PE reduce + epilogue, per psum chunk ---
                        for p0 in range(0, npix_c, pch):
                            ps = pspool.tile([max(32, O), pch], f32,
                                             name=f"ps{li}_{bch}_{p0}", tag="ps")
                            nsl = pch // 512
                            # stream plan per o: DVE o's use PAIRSAD on tile
                            # pairs (+ABSD leftover); ACT o's do all tiles
                            npair = T // 2
                            dve_cost = ((T + 1) // 2) * 1127.0
                            act_cost = T * 1147.0
                            act_frac = dve_cost / (dve_cost + act_cost)
                            acc_frac = 0.0
                            for j in range(32):
                                for q in range(nQ):
                                    o = q * 32 + j
                                    if o >= O:
                                        continue
                                    acc_frac += act_frac
                                    use_act = acc_frac >= 1.0
                                    if use_act:
                                        acc_frac -= 1.0
                                    streams = []
                                    if use_act:
                                        for t in range(T):
                                            rows = pt[t].shape[0]
                                            d = dpool.tile([rows, pch], bf16,
                                                           name=f"d{li}", tag="d")
                                            nc.scalar.activation(
                                                d, pt[t][:, p0:p0 + pch], AF.Abs,
                                                bias=wneg_all[li][t][:, o:o + 1],
                                                scale=1.0)
                                            streams.append((rows, d))
                                    else:
                                        for pi in range(npair):
                                            t0, t1 = 2 * pi, 2 * pi + 1
                                            rows = pt[t0].shape[0]
                                            d = dpool.tile([rows, pch], bf16,
                                                           name=f"dp{li}", tag="d")
                                            nc.vector._custom_dve(
                                                PAIRSAD, out=d[:, :],
                                                in0=pt[t0][:, p0:p0 + pch],
                                                in1=pt[t1][:, p0:p0 + pch],
                                                s0=wpos_all[li][t0][:, o:o + 1],
                                                s1=wpos_all[li][t1][:, o:o + 1])
                                            streams.append((rows, d))
                                        if T % 2:
                                            t0 = T - 1
                                            rows = pt[t0].shape[0]
                                            d = dpool.tile([rows, pch], bf16,
                                                           name=f"ds{li}", tag="d")
                                            nc.vector._custom_dve(
                                                ABSD, out=d[:, :],
                                                in0=pt[t0][:, p0:p0 + pch],
                                                s0=wpos_all[li][t0][:, o:o + 1])
                                            streams.append((rows, d))
                                    nstr = len(streams)
                                    for si, (rows, d) in enumerate(streams):
                                        for sl in range(nsl):
                                            nc.tensor.matmul(
                                                ps[q * 32:q * 32 + 32,
                                                   sl * 512:(sl + 1) * 512],
                                                Tpos[0:rows, 32 - j:64 - j],
                                                d[:, sl * 512:(sl + 1) * 512],
                                                start=(j == 0 and si == 0),
                                                stop=(((j == 31) or (o == O - 1))
                                                      and si == nstr - 1),
                                                tile_position=(0, 32 * q),
                                                skip_group_check=True)
                            # epilogue: relu(-s * psum + b)
                            if pool_after:
                                nc.scalar.activation(
                                    dest[:, p0:p0 + pch], ps[0:O, :], AF.Relu,
                                    bias=bb_all[li][:, :], scale=negs_all[li][:, :])
                            else:
                                Hn = H  # same spatial size, next layer pad Hn+2
                                dv = Ap[li + 1].rearrange(
                                    "c (b h w) -> c b h w", b=BC, h=Hn + 2)
                                # pixel range [p0, p0+pch) within this bchunk:
                                # whole images per chunk (pch % (H*W) == 0)
                                i0 = b0 + p0 // (H * W)
                                ni = pch // (H * W)
                                nc.scalar.activation(
                                    dv[0:O, i0:i0 + ni, 1:H + 1, 1:W + 1],
                                    ps[0:O, :].rearrange(
                                        "c (b h w) -> c b h w", b=ni, h=H),
                                    AF.Relu,
                                    bias=bb_all[li][:, :], scale=negs_all[li][:, :])

                        # --- maxpool 2x2 -> next padded tensor (or A8) ---
                        if pool_after:
                            dv4 = dest.rearrange("c (b h w) -> c b h w", b=bcs, h=H)
                            m1 = tpool.tile([O, npix_c // 2], bf16,
                                            name=f"m1_{li}_{bch}", tag="m1")
                            m1v = m1.rearrange("c (b h w) -> c b h w", b=bcs, h=H)
                            nc.vector.tensor_tensor(
                                m1v, dv4[:, :, :, 0::2], dv4[:, :, :, 1::2], A.max)
                            if li == 7:
                                nxt = A8.rearrange("c (b h w) -> c b h w",
                                                   b=BC, h=4)[0:O, b0:b0 + bcs]
                            else:
                                Hn = H // 2
                                nxt = Ap[li + 1].rearrange(
                                    "c (b h w) -> c b h w", b=BC, h=Hn + 2)[
                                    0:O, b0:b0 + bcs, 1:Hn + 1, 1:Hn + 1]
                            nc.vector.tensor_tensor(
                                nxt, m1v[:, :, 0::2, :], m1v[:, :, 1::2, :], A.max)

            # --- FC: out[cls, b] = sum_c,hw A8[c, b*16+hw] * fcw[c, hw*10+cls] ---
            psf = pspool.tile([32, 512], f32, name="psf", tag="psf", bufs=1)
            for hw in range(16):
                nc.tensor.matmul(
                    psf[0:10, 0:BC],
                    fcw[:, hw * 10:(hw + 1) * 10],
                    bass.AP(tensor=A8.tensor, offset=A8.offset + hw,
                            ap=[list(A8.ap[0]), [16, BC]]),
                    start=(hw == 0), stop=(hw == 15), skip_group_check=True)
            osb = persist.tile([10, BC], f32, name="osb", tag="osb")
            nc.scalar.activation(osb, psf[0:10, 0:BC], AF.Identity,
                                 bias=fcb[:, :], scale=1.0)
            nc.sync.dma_start(
                out=bass.AP(tensor=out_d, offset=0, ap=[[1, 10], [10, BC]]),
                in_=osb)

        if loop_k > 1:
            with tc.For_i(0, loop_k, 1):
                net_body()
        else:
            net_body()

    nc.compile()
    return nc


def _get_nc(cfg=None):
    key = str(sorted((cfg or {}).items()))
    if key not in _CACHE:
        _CACHE[key] = _build(cfg)
    return _CACHE[key]


def make_in_maps(inputs):
    full = dict(inputs)
    x = np.ascontiguousarray(full["x"], dtype=np.float32)
    reps = {k: np.ascontiguousarray(v, np.float32) for k, v in full.items()
            if k != "x"}
    in_maps = []
    for c in range(N_CORES):
        m = {"x": x[c * BC:(c + 1) * BC]}
        m.update(reps)
        in_maps.append(m)
    return in_maps


def kernel(**inputs):
    from concourse import bass_utils
    nc = _get_nc()
    in_maps = make_in_maps(inputs)
    res = bass_utils.run_bass_kernel_spmd(nc, in_maps,
                                          core_ids=list(range(N_CORES)))
    return np.concatenate([r["out"] for r in res.results], axis=0)
